# revision 8
# baseline (speedup 1.0000x reference)
"""AxialAttention Trainium2 kernel v2 (8 NeuronCores, SPMD).

Sharding: core = b*4 + quarter; each core does one batch element and a
10-row H-slab (all 256 channels). The reference's three branches are
numerically identical (h=w=d=40, reshape ignores axis names), so
out = 3 * branch; the 3 is folded into wp, the attention scale into wq/bq.

v2 structural changes vs baseline:
- Cross-deck paired pivots: conv evacuations write q/k/v into padded
  128-col blocks ({deck0: 40, pad 24, deck1: 40, pad 24}), so ONE PE
  transpose per (tensor, w-or-d) produces both decks at partition bands
  0-39/64-103 (the layout scores/AV already want). Halves PE transpose
  cycles (30.7K -> 15.4K per slice).
- Pivot-back moved off PE: o_all [112 part, wq*128+cp] is rearranged to
  branch2 [cp part, wq, 112] by DmaTransposeAnt (XBAR DMA transpose),
  freeing ~4.2K PE cycles/slice and the psum it used.
- Bigger fused evacuations: 2-bank [128, 1024] psum tiles; conv/wp evac
  one op per 800 cols, pivots one op per 1024 cols, scores exp and AV
  evac cover 48 channel-units per op.
- Final output stored bf16 (harness tolerance 2e-2; bf16 adds ~0.2%).
"""

import sys

sys.path.insert(0, "/opt/trn_rl_repo")

import numpy as np
import ml_dtypes
from contextlib import ExitStack

import concourse.bass as bass
import concourse.tile as tile
from concourse import bacc, mybir
from concourse.bass_utils import run_bass_kernel_spmd
from concourse.masks import make_identity

BF16 = mybir.dt.bfloat16
F32 = mybir.dt.float32

B, C, H, W, D = 2, 256, 40, 40, 40
HEADS = 8
HD = C // HEADS
SCALE = HD ** -0.5
N_CORES = 8
SLAB = H // 4           # 10 H-rows per core
WD = W * D              # 1600
NSLAB = SLAB * WD       # 16000
CHALF = 128

IDENT = mybir.ActivationFunctionType.Identity
EXP = mybir.ActivationFunctionType.Exp
MULT = mybir.AluOpType.mult

# attention channel-pair groups: each group = up to 24 column-slots
# (channel-pairs), both decks -> up to 48 units per [128, 1024] psum tile
GROUPS = []
_c0 = 0
while _c0 < CHALF:
    GROUPS.append((_c0, min(12, CHALF - _c0)))
    _c0 += 12


def _merge(a, b):
    """Proportionally interleave two chunk lists, preserving each order."""
    out = []
    na, nb = len(a), len(b)
    ia = ib = 0
    while ia < na or ib < nb:
        if ib >= nb or (ia * (nb + 1) <= ib * (na + 1) and ia < na):
            out.append(a[ia])
            ia += 1
        else:
            out.append(b[ib])
            ib += 1
    return out


def _build_nc():
    nc = bacc.Bacc(
        "TRN2",
        target_bir_lowering=False,
        debug=False,
        num_devices=N_CORES,
    )
    x_d = nc.declare_dram_parameter("x", [C, NSLAB], BF16, isOutput=False)
    wqkv_d = nc.declare_dram_parameter("wqkv", [C, 3 * C], BF16, isOutput=False)
    bqkv_d = nc.declare_dram_parameter("bqkv", [3 * C, 1], F32, isOutput=False)
    wp_d = nc.declare_dram_parameter("wp3", [C, C], BF16, isOutput=False)
    bp_d = nc.declare_dram_parameter("bp", [C, 1], F32, isOutput=False)
    out_d = nc.declare_dram_parameter("out", [C, NSLAB], BF16, isOutput=True)

    with ExitStack() as ctx:
        tc = ctx.enter_context(tile.TileContext(nc))
        const = ctx.enter_context(tc.tile_pool(name="const", bufs=1))
        xp = ctx.enter_context(tc.tile_pool(name="xp", bufs=3))
        qkvp = ctx.enter_context(tc.tile_pool(name="qkvp", bufs=2))
        attp = ctx.enter_context(tc.tile_pool(name="attp", bufs=2))
        ep = ctx.enter_context(tc.tile_pool(name="ep", bufs=3))
        oallp = ctx.enter_context(tc.tile_pool(name="oallp", bufs=2))
        br2p = ctx.enter_context(tc.tile_pool(name="br2p", bufs=2))
        brnp = ctx.enter_context(tc.tile_pool(name="brnp", bufs=2))
        recp = ctx.enter_context(tc.tile_pool(name="recp", bufs=2))
        outp = ctx.enter_context(tc.tile_pool(name="outp", bufs=1))
        # [128, 512] f32 1-bank tiles shared by conv/scores/AV/wp (6 bufs)
        # + [128, 1024] bf16 1-bank transpose tiles (2 bufs)
        ps_a = ctx.enter_context(tc.tile_pool(name="ps_a", bufs=5, space="PSUM"))
        ps_t = ctx.enter_context(tc.tile_pool(name="ps_t", bufs=3, space="PSUM"))

        ident = const.tile([128, 128], BF16)
        make_identity(nc, ident[:])

        def load_x(i):
            x_sb = xp.tile([128, 2, WD], BF16, name="x_sb")
            nc.sync.dma_start(
                x_sb[:],
                x_d.ap()[:, i * WD : (i + 1) * WD].rearrange(
                    "(ko ki) n -> ki ko n", ki=128
                ),
            )
            return x_sb

        # startup DMA order matches the first conv chunk's real deps:
        # x0 n-chunk 0, the q0 weight block and biases, then the remaining
        # x0 n-chunks and weights in the compute shadow
        x_first = xp.tile([128, 2, WD], BF16, name="x_sb")
        x0v = x_d.ap()[:, 0:WD].rearrange("(ko ki) n -> ki ko n", ki=128)
        nc.sync.dma_start(x_first[:, :, 0:400], x0v[:, :, 0:400])
        wqkv_sb = const.tile([128, 2, 3 * C], BF16)
        nc.sync.dma_start(
            wqkv_sb[:, :, 0:128],
            wqkv_d.ap()[:, 0:128].rearrange("(ko ki) m -> ki ko m", ki=128),
        )
        bqkv_sb = const.tile([128, 6, 1], F32)
        nc.sync.dma_start(
            bqkv_sb[:], bqkv_d.ap().rearrange("(mo mi) one -> mi mo one", mi=128)
        )
        for nn in range(1, 4):
            nc.sync.dma_start(
                x_first[:, :, nn * 400 : (nn + 1) * 400],
                x0v[:, :, nn * 400 : (nn + 1) * 400],
            )
        nc.sync.dma_start(
            wqkv_sb[:, :, 128:768],
            wqkv_d.ap()[:, 128:768].rearrange("(ko ki) m -> ki ko m", ki=128),
        )

        def conv_emit(x_sb):
            # qkv_sb padded layout [128, 3 tensors, 40, 128]:
            #   q/k (t=0,1): [t, w, dk*64 + d]  (d-runs per w, both decks)
            #   v   (t=2):   [2, d, dk*64 + w]  (w-runs per d, both decks)
            qkv_sb = qkvp.tile([128, 3, 40, 128], BF16, name="qkv_sb")
            chunks = []
            for m in range(6):          # m = t*2 + dk
                t, dk = m // 2, m % 2
                for n in range(4):
                    def ch(m=m, t=t, dk=dk, n=n, qkv_sb=qkv_sb, x_sb=x_sb):
                        ps = ps_a.tile([128, 512], F32, tag="ps_a", name="conv_ps")
                        for k in range(2):
                            nc.tensor.matmul(
                                ps[:, 0:400],
                                lhsT=wqkv_sb[:, k, m * 128 : (m + 1) * 128],
                                rhs=x_sb[:, k, n * 400 : (n + 1) * 400],
                                start=(k == 0),
                                stop=(k == 1),
                            )
                        src = ps[:, 0:400].rearrange("p (w d) -> p w d", d=40)
                        if t < 2:
                            dst = qkv_sb[
                                :, t, n * 10 : n * 10 + 10, dk * 64 : dk * 64 + 40
                            ]
                        else:
                            dst = qkv_sb[
                                :, 2, :, dk * 64 + n * 10 : dk * 64 + n * 10 + 10
                            ].rearrange("p d w -> p w d")
                        if (m + n) % 2 == 0:
                            nc.vector.tensor_scalar_add(dst, src, bqkv_sb[:, m])
                        else:
                            nc.scalar.activation(
                                out=dst, in_=src, func=IDENT,
                                bias=bqkv_sb[:, m], scale=1.0,
                            )
                    chunks.append(ch)
            return qkv_sb, chunks

        def pivots_emit(qkv_sb):
            q_att = attp.tile([128, W * 128], BF16, tag="q_att", name="q_att")
            k_att = attp.tile([128, W * 128], BF16, tag="k_att", name="k_att")
            v_att = attp.tile([128, 41 * 128], BF16, tag="v_att", name="v_att")
            chunks = [
                lambda: nc.gpsimd.memset(v_att[:, 40 * 128 : 41 * 128], 1.0)
            ]
            for t, dst in ((0, q_att), (1, k_att), (2, v_att)):
                for g in range(5):      # 8 (w or d) per chunk
                    def ch(t=t, g=g, dst=dst):
                        pst = ps_t.tile([128, 1024], BF16, tag="pst", name="pst")
                        for j in range(8):
                            nc.tensor.transpose(
                                pst[:, j * 128 : (j + 1) * 128],
                                qkv_sb[:, t, g * 8 + j, :],
                                ident[:],
                            )
                        dd = dst[0:104, g * 1024 : (g + 1) * 1024]
                        if (t + g) % 2 == 0:
                            nc.vector.tensor_copy(out=dd, in_=pst[0:104, :])
                        else:
                            nc.scalar.copy(dd, pst[0:104, :])
                    chunks.append(ch)
            return (q_att, k_att, v_att), chunks

        def attn_emit(att):
            q_att, k_att, v_att = att
            q_v = q_att.rearrange("p (w c) -> p c w", c=128)
            k_v = k_att.rearrange("p (w c) -> p c w", c=128)
            vv = v_att.rearrange("p (d c) -> p c d", c=128)
            # o_all [112 part (d+den bands), wq*128 + cp]
            o_all = oallp.tile([128, W * 128], BF16, name="o_all")

            def scores_stage(c0, gn):
                s_ps = ps_a.tile([128, 512], F32, tag="ps_a", name="s_ps")
                for j in range(gn):
                    cp = c0 + j
                    for dk in range(2):
                        r0 = dk * 64
                        nc.tensor.matmul(
                            s_ps[r0 : r0 + 40, j * 40 : j * 40 + 40],
                            lhsT=k_v[r0 : r0 + 40, cp],
                            rhs=q_v[r0 : r0 + 40, cp],
                            start=True,
                            stop=True,
                        )
                e_sb = ep.tile([128, 480], BF16, tag="e_sb", name="e_sb")
                nc.scalar.activation(
                    out=e_sb[0:104, 0 : gn * 40],
                    in_=s_ps[0:104, 0 : gn * 40],
                    func=EXP,
                )
                return e_sb

            def av_stage(c0, gn, e_sb, gi=0):
                o_ps = ps_a.tile([128, 512], F32, tag="ps_a", name="o_ps")
                for j in range(gn):
                    cp = c0 + j
                    for dk in range(2):
                        r0 = dk * 64
                        nc.tensor.matmul(
                            o_ps[r0 : r0 + 41, j * 40 : j * 40 + 40],
                            lhsT=vv[r0 : r0 + 40, cp],
                            rhs=e_sb[r0 : r0 + 40, j * 40 : (j + 1) * 40],
                            start=True,
                            stop=True,
                        )
                src = o_ps[0:105, 0 : gn * 40].rearrange("p (j w) -> p j w", w=40)
                dst = o_all.rearrange("p (w c) -> p w c", c=128)[
                    0:105, :, c0 : c0 + gn
                ].rearrange("p w c -> p c w")
                if gi % 2 == 0:
                    nc.vector.tensor_copy(out=dst, in_=src)
                else:
                    nc.scalar.copy(dst, src)

            pend = [None]
            chunks = []
            for gi, (c0, gn) in enumerate(GROUPS):
                def ch(c0=c0, gn=gn, gi=gi):
                    e_sb = scores_stage(c0, gn)
                    if pend[0] is not None:
                        av_stage(*pend[0])
                    pend[0] = (c0, gn, e_sb, gi)
                chunks.append(ch)
            chunks.append(lambda: av_stage(*pend[0]))
            return o_all, chunks

        def pbwp_emit(o_all, i):
            branch2 = br2p.tile([128, W, 112], BF16, name="branch2")
            branch_n = brnp.tile([128, 2, WD], BF16, name="branch_n")
            out_sb = outp.tile([128, 2, WD], BF16, name="out_sb")
            head = []
            # 4 DMA transposes of 10 wq each (issued from SP sequencer)
            for q4 in range(4):
                def ch(q4=q4):
                    nc.sync.dma_start_transpose(
                        branch2[:, q4 * 10 : (q4 + 1) * 10, :],
                        o_all[0:112, q4 * 1280 : (q4 + 1) * 1280],
                    )
                head.append(ch)

            rec = recp.tile([128, 2, W, 1], F32, name="rec")

            def ch_norm():
                # branch_n = branch2 * (1/den) on DVE (deps met by phase 2)
                for dk in range(2):
                    nc.vector.reciprocal(rec[:, dk, :, 0], branch2[:, :, dk * 64 + 40])
                for dk in range(2):
                    nc.vector.tensor_tensor(
                        branch_n[:, dk].rearrange("p (w d) -> p w d", d=40),
                        branch2[:, :, dk * 64 : dk * 64 + 40],
                        rec[:, dk].to_broadcast((128, W, 40)),
                        MULT,
                    )

            chunks = []
            for m in range(2):
                for n in range(4):
                    def ch(m=m, n=n):
                        ps = ps_a.tile([128, 512], F32, tag="ps_a", name="wp_ps")
                        for k in range(2):
                            nc.tensor.matmul(
                                ps[:, 0:400],
                                lhsT=wp_sb[:, k, m * 128 : (m + 1) * 128],
                                rhs=branch_n[:, k, n * 400 : (n + 1) * 400],
                                start=(k == 0),
                                stop=(k == 1),
                            )
                        src = ps[:, 0:400]
                        dst = out_sb[:, m, n * 400 : (n + 1) * 400]
                        if (m + n) % 2 == 0:
                            nc.scalar.activation(
                                out=dst, in_=src, func=IDENT,
                                bias=bp_sb[:, m], scale=1.0,
                            )
                        else:
                            nc.vector.tensor_scalar_add(dst, src, bp_sb[:, m])
                    chunks.append(ch)

            def dma_ch():
                nc.scalar.dma_start(
                    out_d.ap()[:, i * WD : (i + 1) * WD].rearrange(
                        "(ko ki) n -> ki ko n", ki=128
                    ),
                    out_sb[:],
                )
            chunks.append(dma_ch)
            if i < SLAB - 1:
                return head, [], [ch_norm] + chunks

            # final slice: pipeline per wq-quarter (tr -> norm -> wp) to
            # shrink the exposed drain tail
            def norm_q(q4):
                for dk in range(2):
                    nc.vector.reciprocal(
                        rec[:, dk, q4 * 10 : (q4 + 1) * 10, 0],
                        branch2[:, q4 * 10 : (q4 + 1) * 10, dk * 64 + 40],
                    )
                for dk in range(2):
                    nc.vector.tensor_tensor(
                        branch_n[:, dk].rearrange("p (w d) -> p w d", d=40)[
                            :, q4 * 10 : (q4 + 1) * 10
                        ],
                        branch2[:, q4 * 10 : (q4 + 1) * 10, dk * 64 : dk * 64 + 40],
                        rec[:, dk, q4 * 10 : (q4 + 1) * 10].to_broadcast(
                            (128, 10, 40)
                        ),
                        MULT,
                    )

            def store_q(q4):
                # flush this wq-quarter (both m halves) immediately
                nc.scalar.dma_start(
                    out_d.ap()[:, i * WD : (i + 1) * WD]
                    .rearrange("(ko ki) n -> ki ko n", ki=128)[
                        :, :, q4 * 400 : (q4 + 1) * 400
                    ],
                    out_sb[:, :, q4 * 400 : (q4 + 1) * 400],
                )

            seq = [head[0], head[1]]
            for q4 in range(4):
                if q4 + 2 < 4:
                    seq.append(head[q4 + 2])
                seq.append(lambda q4=q4: norm_q(q4))
                seq.append(chunks[q4])          # wp m=0, n=q4
                seq.append(chunks[4 + q4])      # wp m=1, n=q4
                seq.append(lambda q4=q4: store_q(q4))
            return [], [], seq

        # ---- software-pipelined slice loop
        x_cur = x_first
        qkv_cur, conv_ch = conv_emit(x_cur)
        for ch in conv_ch:
            ch()
        # wp/bp are not needed until the first wp conv, a full slice later
        wp_sb = const.tile([128, 2, C], BF16)
        nc.scalar.dma_start(
            wp_sb[:], wp_d.ap().rearrange("(ko ki) m -> ki ko m", ki=128)
        )
        bp_sb = const.tile([128, 2, 1], F32)
        nc.scalar.dma_start(
            bp_sb[:], bp_d.ap().rearrange("(mo mi) one -> mi mo one", mi=128)
        )
        x_pre = {0: x_cur}
        x_pre[1] = load_x(1)
        pending_head, pending_norm, pending_tail = [], [], []
        for i in range(SLAB):
            if i + 2 < SLAB:
                x_pre[i + 2] = load_x(i + 2)
            x_nxt = x_pre.get(i + 1)
            # pbwp(i-1) head: DMA transposes
            for ch in pending_head:
                ch()
            att, piv_ch = pivots_emit(qkv_cur)
            if i + 1 < SLAB:
                qkv_nxt, conv_ch = conv_emit(x_nxt)
            else:
                qkv_nxt, conv_ch = None, []
            # phase 1: pivots(i) + conv(i+1); normalize(i-1) mid-phase so
            # its DMA-transpose deps are met and Pool finishes before wp
            lst2 = conv_ch[:12] + pending_norm + conv_ch[12:]
            for ch in _merge(piv_ch, lst2):
                ch()
            o_all, attn_ch = attn_emit(att)
            # phase 2: attention(i) with the normalize divide slotted mid-
            # stream (so DVE finishes it early), then wp(i-1) tail
            for ch in attn_ch:
                ch()
            for ch in pending_tail:
                ch()
            pending_head, pending_norm, pending_tail = pbwp_emit(o_all, i)
            qkv_cur = qkv_nxt
        for ch in pending_head:
            ch()
        for ch in pending_norm:
            ch()
        for ch in pending_tail:
            ch()

    nc.compile()
    return nc


_NC_CACHE = None


def _get_nc():
    global _NC_CACHE
    if _NC_CACHE is None:
        _NC_CACHE = _build_nc()
    return _NC_CACHE


def make_in_maps(x, wq, bq, wk, bk, wv, bv, wp, bp):
    bf = ml_dtypes.bfloat16
    wqkv = np.concatenate(
        [wq.T * SCALE, wk.T, wv.T], axis=1
    ).astype(bf)  # [C, 3C], lhsT layout (c_in rows, c_out cols)
    bqkv = np.concatenate([bq * SCALE, bk, bv]).reshape(3 * C, 1).astype(np.float32)
    wp3 = (3.0 * wp).T.astype(bf)  # [C, C]
    bp_ = bp.reshape(C, 1).astype(np.float32)
    in_maps = []
    for core in range(N_CORES):
        b = core // 4
        r0 = (core % 4) * SLAB
        x_slab = np.ascontiguousarray(
            x[b, :, r0 : r0 + SLAB].reshape(C, NSLAB)
        ).astype(bf)
        in_maps.append(
            {"x": x_slab, "wqkv": wqkv, "bqkv": bqkv, "wp3": wp3, "bp": bp_}
        )
    return in_maps


def run_on_cores(in_maps, **kw):
    nc = _get_nc()
    return run_bass_kernel_spmd(nc, in_maps, core_ids=list(range(N_CORES)), **kw)


def kernel(x, wq, bq, wk, bk, wv, bv, wp, bp):
    x = np.asarray(x, dtype=np.float32)
    in_maps = make_in_maps(
        x,
        np.asarray(wq, np.float32),
        np.asarray(bq, np.float32),
        np.asarray(wk, np.float32),
        np.asarray(bk, np.float32),
        np.asarray(wv, np.float32),
        np.asarray(bv, np.float32),
        np.asarray(wp, np.float32),
        np.asarray(bp, np.float32),
    )
    res = run_on_cores(in_maps)
    out = np.empty((B, C, H, W, D), np.float32)
    for core in range(N_CORES):
        b = core // 4
        r0 = (core % 4) * SLAB
        out[b, :, r0 : r0 + SLAB] = (
            res.results[core]["out"].astype(np.float32).reshape(C, SLAB, W, D)
        )
    return out


if __name__ == "__main__":
    rng = np.random.default_rng(0)
    ins = {
        "x": rng.standard_normal((B, C, H, W, D), np.float32),
        "wq": rng.standard_normal((C, C), np.float32) / 16,
        "bq": rng.standard_normal(C).astype(np.float32) * 0.01,
        "wk": rng.standard_normal((C, C), np.float32) / 16,
        "bk": rng.standard_normal(C).astype(np.float32) * 0.01,
        "wv": rng.standard_normal((C, C), np.float32) / 16,
        "bv": rng.standard_normal(C).astype(np.float32) * 0.01,
        "wp": rng.standard_normal((C, C), np.float32) / 16,
        "bp": rng.standard_normal(C).astype(np.float32) * 0.01,
    }
    out = kernel(**ins)
    print("kernel ran, out shape", out.shape, "mean", float(np.abs(out).mean()))
